# revision 5
# baseline (speedup 1.0000x reference)
"""Trainium2 Bass kernel for AdvancedCausalSelfAttention (GQA + QK-RMSNorm + RoPE + q_gain).

Problem shape (hardcoded): B=4, S=2048, D=2048, 16 q heads / 4 kv heads, head_dim=128.
Sharding over 8 NeuronCores: core c = (batch b = c//2, head-group g = c%2).
Each core computes, for its batch, the attention output of 8 q heads (2 kv heads)
and the partial O-projection out_partial = y_g @ Wo[:, g*1024:(g+1)*1024].T.
Host sums the two partials per batch. No collectives.

Compute dtype: bf16 matmuls (f32 PSUM accumulation), f32 softmax/norm statistics.
"""
import math

import numpy as np
import ml_dtypes

import concourse.bass as bass
import concourse.mybir as mybir
import concourse.tile as tile
from concourse import bacc
from concourse.masks import make_identity

F32 = mybir.dt.float32
BF16 = mybir.dt.bfloat16
BF16NP = ml_dtypes.bfloat16

BSZ, SEQ, DIM = 4, 2048, 2048
NH, NKV, HD = 16, 4, 128
HPG = NH // 2          # 8 q heads per group
KPG = NKV // 2         # 2 kv heads per group
TC = SEQ // 128        # 16 query/key tiles
DC = DIM // 128        # 16 contraction chunks
N_CORES = 8
ROPE_BASE = 10000.0
EPS = float(np.finfo(np.float32).eps)
SCALE = 1.0 / math.sqrt(HD)
MASKVAL = -1e9


def build_nc(loop_iters: int | None = None):
    """Build + finalize the per-core Bass graph. `loop_iters` wraps the whole
    body in a hardware loop (for timing); None = straight-line single pass."""
    nc = bacc.Bacc()

    xt = nc.declare_dram_parameter("xt", [TC, 128, DC, 128], BF16, isOutput=False)
    wq = nc.declare_dram_parameter("wq", [DC, 128, HPG * HD], BF16, isOutput=False)
    wk = nc.declare_dram_parameter("wk", [DC, 128, KPG * HD], BF16, isOutput=False)
    wv = nc.declare_dram_parameter("wv", [DC, 128, KPG * HD], BF16, isOutput=False)
    wo = nc.declare_dram_parameter("wo", [HPG, 128, DIM], BF16, isOutput=False)
    cs = nc.declare_dram_parameter("cs", [TC, 128, 64], F32, isOutput=False)
    sn = nc.declare_dram_parameter("sn", [TC, 128, 64], F32, isOutput=False)
    gsc = nc.declare_dram_parameter("gsc", [128, HPG], F32, isOutput=False)
    cmt = nc.declare_dram_parameter("cmt", [128, 128], F32, isOutput=False)
    tok = nc.declare_dram_parameter("tok", [1, 1], F32, isOutput=False)

    out = nc.declare_dram_parameter("out", [SEQ, DIM], F32, isOutput=True)
    tok_out = nc.declare_dram_parameter("tok_out", [1, 1], F32, isOutput=True)

    out_r = out.rearrange("(tc p) o -> tc p o", p=128)

    def bcast(ap, n, axis):
        """Insert a step-0 dim of size n at free-axis position `axis` (1-based incl partition)."""
        new_ap = list(ap.ap)
        new_ap.insert(axis, [0, n])
        return bass.AP(tensor=ap.tensor, offset=ap.offset, ap=new_ap)

    with tile.TileContext(nc) as tc_:
        body_ctx = tc_.For_i(0, loop_iters, 1) if loop_iters is not None else None
        if body_ctx is not None:
            body_ctx.__enter__()
        with tc_.tile_pool(name="const", bufs=1) as constp, \
             tc_.tile_pool(name="store", bufs=1) as store:
            # constants
            tok_t = constp.tile([1, 1], F32)
            nc.sync.dma_start(out=tok_t, in_=tok[:])
            nc.sync.dma_start(out=tok_out[:], in_=tok_t)
            ident = constp.tile([128, 128], BF16)
            make_identity(nc, ident[:])
            ones_t = constp.tile([128, 1], BF16)
            nc.vector.memset(ones_t[:], 1.0)
            cmt_t = constp.tile([128, 128], F32)
            nc.sync.dma_start(out=cmt_t, in_=cmt[:])
            gsc_t = constp.tile([128, HPG], F32)
            nc.sync.dma_start(out=gsc_t, in_=gsc[:])
            eps_t = constp.tile([128, 1], F32)
            nc.vector.memset(eps_t[:], EPS)

            # persistent stores (bf16)
            qT = store.tile([128, HPG, TC, 128], BF16)   # [d, h, tc, t]
            kT = store.tile([128, KPG, TC, 128], BF16)   # [d, kvh, tc, t]
            vS = store.tile([128, TC, KPG * HD], BF16)   # [kv, tc, kvh*128+d]
            yT = store.tile([128, HPG, TC, 128], BF16)   # [d, h(cc), tc, t]

            # ---------------- Phase A: QKV projection + norm + rope + transpose
            with tc_.tile_pool(name="wqkv", bufs=1) as wp, \
                 tc_.tile_pool(name="xs", bufs=3) as xs, \
                 tc_.tile_pool(name="csp", bufs=2) as csp, \
                 tc_.tile_pool(name="scrA", bufs=2) as scr, \
                 tc_.tile_pool(name="psA", bufs=2, space="PSUM") as psA, \
                 tc_.tile_pool(name="psA1", bufs=1, space="PSUM") as psA1:
                wq_t = wp.tile([128, DC, HPG * HD], BF16)
                nc.sync.dma_start(out=wq_t, in_=wq.rearrange("dc p o -> p dc o"))
                wk_t = wp.tile([128, DC, KPG * HD], BF16)
                nc.sync.dma_start(out=wk_t, in_=wk.rearrange("dc p o -> p dc o"))
                wv_t = wp.tile([128, DC, KPG * HD], BF16)
                nc.sync.dma_start(out=wv_t, in_=wv.rearrange("dc p o -> p dc o"))

                for tci in range(TC):
                    x_t = xs.tile([128, DC, 128], BF16, tag="x")
                    nc.sync.dma_start(out=x_t, in_=xt[tci])
                    cos_t = csp.tile([128, 64], F32, tag="cos")
                    nc.sync.dma_start(out=cos_t, in_=cs[tci])
                    sin_t = csp.tile([128, 64], F32, tag="sin")
                    nc.sync.dma_start(out=sin_t, in_=sn[tci])

                    q_ps = psA.tile([128, HPG * HD], F32, tag="qp")
                    k_ps = psA1.tile([128, KPG * HD], F32, tag="kp")
                    v_ps = psA1.tile([128, KPG * HD], F32, tag="vp")
                    for dc in range(DC):
                        st = dc == 0
                        sp = dc == DC - 1
                        nc.tensor.matmul(q_ps[:, 0:512], x_t[:, dc], wq_t[:, dc, 0:512], start=st, stop=sp)
                        nc.tensor.matmul(q_ps[:, 512:1024], x_t[:, dc], wq_t[:, dc, 512:1024], start=st, stop=sp)
                        nc.tensor.matmul(k_ps[:], x_t[:, dc], wk_t[:, dc], start=st, stop=sp)
                        nc.tensor.matmul(v_ps[:], x_t[:, dc], wv_t[:, dc], start=st, stop=sp)

                    # V: straight evac (cast bf16)
                    nc.scalar.copy(vS[:, tci, :], v_ps[:])

                    # RMS stats via Square activation with free-axis accumulate
                    q_ps3 = q_ps.rearrange("p (h d) -> p h d", h=HPG)
                    k_ps3 = k_ps.rearrange("p (h d) -> p h d", h=KPG)
                    qss = scr.tile([128, HPG], F32, tag="qss")
                    kss = scr.tile([128, KPG], F32, tag="kss")
                    sqdump = scr.tile([128, 128], BF16, tag="sqdump")
                    for h in range(HPG):
                        nc.scalar.activation(out=sqdump[:], in_=q_ps3[:, h],
                                             func=mybir.ActivationFunctionType.Square,
                                             accum_out=qss[:, h:h + 1])
                    for h in range(KPG):
                        nc.scalar.activation(out=sqdump[:], in_=k_ps3[:, h],
                                             func=mybir.ActivationFunctionType.Square,
                                             accum_out=kss[:, h:h + 1])
                    # rstd = 1/sqrt(ms + eps); fold gain & 1/sqrt(hd) into q scale
                    qstd = scr.tile([128, HPG], F32, tag="qstd")
                    nc.scalar.activation(out=qstd[:], in_=qss[:],
                                         func=mybir.ActivationFunctionType.Sqrt,
                                         scale=1.0 / HD, bias=eps_t[:])
                    qsc = scr.tile([128, HPG], F32, tag="qsc")
                    nc.vector.reciprocal(qsc[:], qstd[:])
                    nc.vector.tensor_mul(qsc[:], qsc[:], gsc_t[:])
                    kstd = scr.tile([128, KPG], F32, tag="kstd")
                    nc.scalar.activation(out=kstd[:], in_=kss[:],
                                         func=mybir.ActivationFunctionType.Sqrt,
                                         scale=1.0 / HD, bias=eps_t[:])
                    ksc = scr.tile([128, KPG], F32, tag="ksc")
                    nc.vector.reciprocal(ksc[:], kstd[:])

                    # scale (broadcast over head_dim), f32 SBUF staging
                    qs = scr.tile([128, HPG, HD], F32, tag="qs")
                    nc.vector.tensor_mul(qs[:], q_ps3, bcast(qsc[:], HD, 2))
                    ks = scr.tile([128, KPG, HD], F32, tag="ks")
                    nc.vector.tensor_mul(ks[:], k_ps3, bcast(ksc[:], HD, 2))

                    # rope (batched over heads; cos/sin broadcast over h)
                    cos_bq = bcast(cos_t[:], HPG, 1)
                    sin_bq = bcast(sin_t[:], HPG, 1)
                    cos_bk = bcast(cos_t[:], KPG, 1)
                    sin_bk = bcast(sin_t[:], KPG, 1)
                    t1 = scr.tile([128, HPG, 64], F32, tag="t1")
                    t2 = scr.tile([128, HPG, 64], F32, tag="t2")
                    qr = scr.tile([128, HPG, 2, 64], BF16, tag="qr")
                    q1 = qs[:, :, 0:64]
                    q2 = qs[:, :, 64:128]
                    nc.vector.tensor_mul(t1[:], q1, cos_bq)
                    nc.vector.tensor_mul(t2[:], q2, sin_bq)
                    nc.vector.tensor_add(qr[:, :, 0, :], t1[:], t2[:])
                    nc.vector.tensor_mul(t1[:], q2, cos_bq)
                    nc.vector.tensor_mul(t2[:], q1, sin_bq)
                    nc.vector.tensor_sub(qr[:, :, 1, :], t1[:], t2[:])
                    k1 = ks[:, :, 0:64]
                    k2 = ks[:, :, 64:128]
                    t1k = scr.tile([128, KPG, 64], F32, tag="t1k")
                    t2k = scr.tile([128, KPG, 64], F32, tag="t2k")
                    kr = scr.tile([128, KPG, 2, 64], BF16, tag="kr")
                    nc.vector.tensor_mul(t1k[:], k1, cos_bk)
                    nc.vector.tensor_mul(t2k[:], k2, sin_bk)
                    nc.vector.tensor_add(kr[:, :, 0, :], t1k[:], t2k[:])
                    nc.vector.tensor_mul(t1k[:], k2, cos_bk)
                    nc.vector.tensor_mul(t2k[:], k1, sin_bk)
                    nc.vector.tensor_sub(kr[:, :, 1, :], t1k[:], t2k[:])

                    # transpose q/k tiles into [d, t] stores
                    for h in range(HPG):
                        tq_ps = psA.tile([128, 128], BF16, tag="trp")
                        nc.tensor.transpose(tq_ps[:], qr[:, h].rearrange("p a b -> p (a b)"), ident[:])
                        if h % 2 == 0:
                            nc.scalar.copy(qT[:, h, tci, :], tq_ps[:])
                        else:
                            nc.vector.tensor_copy(qT[:, h, tci, :], tq_ps[:])
                    for h in range(KPG):
                        tk_ps = psA.tile([128, 128], BF16, tag="trp")
                        nc.tensor.transpose(tk_ps[:], kr[:, h].rearrange("p a b -> p (a b)"), ident[:])
                        if h % 2 == 0:
                            nc.scalar.copy(kT[:, h, tci, :], tk_ps[:])
                        else:
                            nc.vector.tensor_copy(kT[:, h, tci, :], tk_ps[:])

            # ---------------- Phases B (flash attention) + C (O-projection)
            with tc_.tile_pool(name="wo", bufs=1) as wop:
                wo_t = wop.tile([128, HPG, DIM], BF16)
                nc.sync.dma_start(out=wo_t, in_=wo.rearrange("cc p o -> p cc o"))

                with tc_.tile_pool(name="ptp", bufs=3) as ptp, \
                     tc_.tile_pool(name="scrB", bufs=2) as scrB, \
                     tc_.tile_pool(name="psB", bufs=2, space="PSUM") as psB, \
                     tc_.tile_pool(name="psB1", bufs=1, space="PSUM") as psB1:
                    for h in range(HPG):
                        kvh = h // (NH // NKV)
                        for ti in range(TC):
                            nj = ti + 1
                            y_ps = psB.tile([128, 128], F32, tag="y")
                            sum_ps = psB1.tile([128, 1], F32, tag="sum")
                            for jb in range(0, nj, 8):
                                jbe = min(jb + 8, nj)
                                w = (jbe - jb) * 128
                                s_ps = psB.tile([128, 1024], F32, tag="s8")
                                for j in range(jb, jbe):
                                    c0 = (j - jb) * 128
                                    nc.tensor.matmul(s_ps[:, c0:c0 + 128], kT[:, kvh, j, :],
                                                     qT[:, h, ti, :], start=True, stop=True)
                                    if j == ti:
                                        nc.vector.tensor_add(s_ps[:, c0:c0 + 128],
                                                             s_ps[:, c0:c0 + 128], cmt_t[:])
                                pt = ptp.tile([128, 1024], BF16, tag="pt")
                                nc.scalar.activation(out=pt[:, 0:w], in_=s_ps[:, 0:w],
                                                     func=mybir.ActivationFunctionType.Exp)
                                for j in range(jb, jbe):
                                    c0 = (j - jb) * 128
                                    nc.tensor.matmul(y_ps[:], pt[:, c0:c0 + 128],
                                                     vS[:, j, kvh * HD:(kvh + 1) * HD],
                                                     start=(j == 0), stop=(j == ti))
                                    nc.tensor.matmul(sum_ps[:], pt[:, c0:c0 + 128], ones_t[:],
                                                     start=(j == 0), stop=(j == ti))
                            rinv = scrB.tile([128, 1], F32, tag="rinv")
                            nc.vector.reciprocal(rinv[:], sum_ps[:])
                            y_sb = scrB.tile([128, 128], BF16, tag="ysb")
                            nc.vector.tensor_scalar_mul(y_sb[:], y_ps[:], rinv[:])
                            yt_ps = psB1.tile([128, 128], BF16, tag="ytr")
                            nc.tensor.transpose(yt_ps[:], y_sb[:], ident[:])
                            nc.scalar.copy(yT[:, h, ti, :], yt_ps[:])

                # Phase C
                with tc_.tile_pool(name="oC", bufs=2) as oC, \
                     tc_.tile_pool(name="psC", bufs=2, space="PSUM") as psC:
                    for ti in range(TC):
                        o_ps = psC.tile([128, DIM], F32, tag="op")
                        for cc in range(HPG):
                            for oc in range(4):
                                nc.tensor.matmul(o_ps[:, oc * 512:(oc + 1) * 512],
                                                 yT[:, cc, ti, :],
                                                 wo_t[:, cc, oc * 512:(oc + 1) * 512],
                                                 start=(cc == 0), stop=(cc == HPG - 1))
                        o_sb = oC.tile([128, DIM], F32, tag="ob")
                        nc.scalar.copy(o_sb[:, 0:1024], o_ps[:, 0:1024])
                        nc.vector.tensor_copy(o_sb[:, 1024:2048], o_ps[:, 1024:2048])
                        nc.sync.dma_start(out=out_r[ti], in_=o_sb)
        if body_ctx is not None:
            body_ctx.__exit__(None, None, None)

    nc.finalize()
    return nc


# ---------------------------------------------------------------- host side

def _rope_tables():
    inv_freq = ROPE_BASE ** (-np.arange(0, HD, 2, dtype=np.float32) / HD)
    t = np.arange(SEQ, dtype=np.float32)
    freqs = t[:, None] * inv_freq[None, :]
    return np.cos(freqs).astype(np.float32), np.sin(freqs).astype(np.float32)


def make_in_maps(x, Wq, Wk, Wv, Wo, q_gain):
    x = np.asarray(x, np.float32)
    Wq = np.asarray(Wq, np.float32)
    Wk = np.asarray(Wk, np.float32)
    Wv = np.asarray(Wv, np.float32)
    Wo = np.asarray(Wo, np.float32)
    q_gain = np.asarray(q_gain, np.float32)

    cos, sin = _rope_tables()
    cs = np.ascontiguousarray(cos.reshape(TC, 128, 64))
    sn = np.ascontiguousarray(sin.reshape(TC, 128, 64))
    r = np.arange(128)
    cmt = np.where(r[:, None] <= r[None, :], 0.0, MASKVAL).astype(np.float32)
    tokz = np.zeros((1, 1), np.float32)

    xts = []
    for b in range(BSZ):
        X5 = x[b].reshape(TC, 128, DC, 128)          # [tc, tt, dc, p]
        xts.append(np.ascontiguousarray(X5.transpose(0, 3, 2, 1)).astype(BF16NP))
    gw = []
    for g in range(2):
        wq_g = np.ascontiguousarray(Wq[g * 1024:(g + 1) * 1024].T).astype(BF16NP).reshape(DC, 128, 1024)
        wk_g = np.ascontiguousarray(Wk[g * 256:(g + 1) * 256].T).astype(BF16NP).reshape(DC, 128, 256)
        wv_g = np.ascontiguousarray(Wv[g * 256:(g + 1) * 256].T).astype(BF16NP).reshape(DC, 128, 256)
        wo_g = np.ascontiguousarray(Wo[:, g * 1024:(g + 1) * 1024].T).astype(BF16NP).reshape(HPG, 128, DIM)
        gsc_g = np.tile((q_gain[g * HPG:(g + 1) * HPG] * SCALE)[None, :], (128, 1)).astype(np.float32)
        gw.append((wq_g, wk_g, wv_g, wo_g, gsc_g))

    in_maps = []
    for c in range(N_CORES):
        b, g = c // 2, c % 2
        wq_g, wk_g, wv_g, wo_g, gsc_g = gw[g]
        in_maps.append(dict(xt=xts[b], wq=wq_g, wk=wk_g, wv=wv_g, wo=wo_g,
                            cs=cs, sn=sn, gsc=gsc_g, cmt=cmt, tok=tokz))
    return in_maps


# cached compiled runner --------------------------------------------------

_STATE = {}


def _get_runner(loop_iters=None):
    key = loop_iters
    if key in _STATE:
        return _STATE[key]
    import jax
    from jax.sharding import Mesh, PartitionSpec
    try:
        from jax.experimental.shard_map import shard_map
    except ImportError:
        from jax.shard_map import shard_map
    from concourse import bass2jax
    from concourse.bass2jax import _bass_exec_p, install_neuronx_cc_hook, partition_id_tensor

    install_neuronx_cc_hook()
    nc = build_nc(loop_iters)

    partition_name = nc.partition_id_tensor.name if nc.partition_id_tensor else None
    in_names, out_names, out_avals = [], [], []
    for alloc in nc.m.functions[0].allocations:
        if not isinstance(alloc, mybir.MemoryLocationSet):
            continue
        name = alloc.memorylocations[0].name
        if alloc.kind == "ExternalInput":
            if name != partition_name:
                in_names.append(name)
        elif alloc.kind == "ExternalOutput":
            out_names.append(name)
            out_avals.append(jax.core.ShapedArray(tuple(alloc.tensor_shape),
                                                  mybir.dt.np(alloc.dtype)))
    n_params = len(in_names)
    all_in = list(in_names + out_names)
    if partition_name is not None:
        all_in.append(partition_name)
    all_in = tuple(all_in)

    def _body(*args):
        operands = list(args)
        if partition_name is not None:
            operands.append(partition_id_tensor())
        outs = _bass_exec_p.bind(
            *operands,
            out_avals=tuple(out_avals),
            in_names=all_in,
            out_names=tuple(out_names),
            lowering_input_output_aliases=(),
            sim_require_finite=True,
            sim_require_nnan=True,
            nc=nc,
        )
        return tuple(outs)

    devices = jax.devices()[:N_CORES]
    mesh = Mesh(np.asarray(devices), ("core",))
    specs = (PartitionSpec("core"),)
    f = jax.jit(shard_map(_body, mesh=mesh,
                          in_specs=specs * (n_params + len(out_names)),
                          out_specs=specs * len(out_names),
                          check_rep=False),
                keep_unused=True)
    zero_shapes = [(tuple(a.shape), a.dtype) for a in out_avals]
    st = dict(f=f, in_names=in_names, out_names=out_names, zero_shapes=zero_shapes)
    _STATE[key] = st
    return st


def run_on_device(in_maps, loop_iters=None, device_args=None):
    """Returns (out_arrays_by_name, device_args) — device_args reusable for re-runs."""
    st = _get_runner(loop_iters)
    if device_args is None:
        concat = [np.concatenate([m[n] for m in in_maps], axis=0) for n in st["in_names"]]
        zeros = [np.zeros((N_CORES * s[0],) + tuple(s[1:]), d) for s, d in st["zero_shapes"]]
        device_args = concat + zeros
    outs = st["f"](*device_args)
    return outs, device_args


def kernel(x, Wq, Wk, Wv, Wo, q_gain):
    in_maps = make_in_maps(x, Wq, Wk, Wv, Wo, q_gain)
    outs, _ = run_on_device(in_maps)
    st = _get_runner(None)
    oidx = st["out_names"].index("out")
    full = np.asarray(outs[oidx]).reshape(N_CORES, SEQ, DIM)
    result = np.empty((BSZ, SEQ, DIM), np.float32)
    for b in range(BSZ):
        result[b] = full[2 * b] + full[2 * b + 1]
    return result


# revision 8
# speedup vs baseline: 2.2613x; 2.2613x over previous
"""Trainium2 Bass kernel for AdvancedCausalSelfAttention (GQA + QK-RMSNorm + RoPE + q_gain).

Problem shape (hardcoded): B=4, S=2048, D=2048, 16 q heads / 4 kv heads, head_dim=128.
Sharding over 8 NeuronCores: core c = (batch b = c//2, head-group g = c%2).
Each core computes, for its batch, the attention output of 8 q heads (2 kv heads)
and the partial O-projection out_partial = y_g @ Wo[:, g*1024:(g+1)*1024].T.
Host sums the two partials per batch. No collectives.

Compute dtype: bf16 matmuls (f32 PSUM accumulation), f32 softmax/norm statistics.
"""
import math

import numpy as np
import ml_dtypes

import concourse.bass as bass
import concourse.mybir as mybir
import concourse.tile as tile
from concourse import bacc
from concourse.masks import make_identity

F32 = mybir.dt.float32
BF16 = mybir.dt.bfloat16
BF16NP = ml_dtypes.bfloat16

BSZ, SEQ, DIM = 4, 2048, 2048
NH, NKV, HD = 16, 4, 128
HPG = NH // 2          # 8 q heads per group
KPG = NKV // 2         # 2 kv heads per group
TC = SEQ // 128        # 16 query/key tiles
DC = DIM // 128        # 16 contraction chunks
N_CORES = 8
ROPE_BASE = 10000.0
EPS = float(np.finfo(np.float32).eps)
SCALE = 1.0 / math.sqrt(HD)
MASKVAL = -1e9


def build_nc(loop_iters: int | None = None):
    """Build + finalize the per-core Bass graph. `loop_iters` wraps the whole
    body in a hardware loop (for timing); None = straight-line single pass."""
    nc = bacc.Bacc()

    xt = nc.declare_dram_parameter("xt", [TC, 128, DC, 128], BF16, isOutput=False)
    wq = nc.declare_dram_parameter("wq", [DC, 128, HPG * HD], BF16, isOutput=False)
    wk = nc.declare_dram_parameter("wk", [DC, 128, KPG * HD], BF16, isOutput=False)
    wv = nc.declare_dram_parameter("wv", [DC, 128, KPG * HD], BF16, isOutput=False)
    wo = nc.declare_dram_parameter("wo", [HPG, 128, DIM], BF16, isOutput=False)
    cs = nc.declare_dram_parameter("cs", [TC, 128, 64], F32, isOutput=False)
    sn = nc.declare_dram_parameter("sn", [TC, 128, 64], F32, isOutput=False)
    gsc = nc.declare_dram_parameter("gsc", [128, HPG], F32, isOutput=False)
    tok = nc.declare_dram_parameter("tok", [1, 1], F32, isOutput=False)

    out = nc.declare_dram_parameter("out", [SEQ, DIM], F32, isOutput=True)
    tok_out = nc.declare_dram_parameter("tok_out", [1, 1], F32, isOutput=True)

    out_r = out.rearrange("(tc p) o -> tc p o", p=128)

    def bcast(ap, n, axis):
        """Insert a step-0 dim of size n at free-axis position `axis` (1-based incl partition)."""
        new_ap = list(ap.ap)
        new_ap.insert(axis, [0, n])
        return bass.AP(tensor=ap.tensor, offset=ap.offset, ap=new_ap)

    with tile.TileContext(nc) as tc_:
        body_ctx = tc_.For_i(0, loop_iters, 1) if loop_iters is not None else None
        if body_ctx is not None:
            body_ctx.__enter__()
        with tc_.tile_pool(name="const", bufs=1) as constp, \
             tc_.tile_pool(name="store", bufs=1) as store:
            # constants
            tok_t = constp.tile([1, 1], F32)
            nc.sync.dma_start(out=tok_t, in_=tok[:])
            nc.sync.dma_start(out=tok_out[:], in_=tok_t)
            ident = constp.tile([128, 128], BF16)
            make_identity(nc, ident[:])
            gsc_t = constp.tile([128, HPG], F32)
            nc.sync.dma_start(out=gsc_t, in_=gsc[:])
            eps_t = constp.tile([128, 1], F32)
            nc.vector.memset(eps_t[:], EPS)

            # persistent stores (bf16)
            qT = store.tile([128, HPG, TC, 128], BF16)   # [d, h, tc, t]
            kT = store.tile([128, KPG, TC, 128], BF16)   # [d, kvh, tc, t]
            vS = store.tile([128, TC, KPG, HD + 1], BF16)  # [kv, tc, kvh, d | ones]
            yT = store.tile([128, HPG, TC, 128], BF16)   # [d, h(cc), tc, t]
            nc.vector.memset(vS[:, :, :, HD:HD + 1], 1.0)

            # ---------------- Phase A: QKV projection + norm + rope + transpose
            with tc_.tile_pool(name="wqkv", bufs=1) as wp, \
                 tc_.tile_pool(name="xs", bufs=3) as xs, \
                 tc_.tile_pool(name="csp", bufs=2) as csp, \
                 tc_.tile_pool(name="scrA", bufs=2) as scr, \
                 tc_.tile_pool(name="psA", bufs=2, space="PSUM") as psA, \
                 tc_.tile_pool(name="psA1", bufs=1, space="PSUM") as psA1:
                wq_t = wp.tile([128, DC, HPG * HD], BF16)
                nc.sync.dma_start(out=wq_t, in_=wq.rearrange("dc p o -> p dc o"))
                wk_t = wp.tile([128, DC, KPG * HD], BF16)
                nc.sync.dma_start(out=wk_t, in_=wk.rearrange("dc p o -> p dc o"))
                wv_t = wp.tile([128, DC, KPG * HD], BF16)
                nc.sync.dma_start(out=wv_t, in_=wv.rearrange("dc p o -> p dc o"))

                for tci in range(TC):
                    x_t = xs.tile([128, DC, 128], BF16, tag="x")
                    nc.sync.dma_start(out=x_t, in_=xt[tci])
                    cos_t = csp.tile([128, 64], F32, tag="cos")
                    nc.sync.dma_start(out=cos_t, in_=cs[tci])
                    sin_t = csp.tile([128, 64], F32, tag="sin")
                    nc.sync.dma_start(out=sin_t, in_=sn[tci])

                    q_ps = psA.tile([128, HPG * HD], F32, tag="qp")
                    k_ps = psA1.tile([128, KPG * HD], F32, tag="kp")
                    v_ps = psA1.tile([128, KPG * HD], F32, tag="vp")
                    for dc in range(DC):
                        st = dc == 0
                        sp = dc == DC - 1
                        nc.tensor.matmul(q_ps[:, 0:512], x_t[:, dc], wq_t[:, dc, 0:512], start=st, stop=sp)
                        nc.tensor.matmul(q_ps[:, 512:1024], x_t[:, dc], wq_t[:, dc, 512:1024], start=st, stop=sp)
                        nc.tensor.matmul(k_ps[:], x_t[:, dc], wk_t[:, dc], start=st, stop=sp)
                        nc.tensor.matmul(v_ps[:], x_t[:, dc], wv_t[:, dc], start=st, stop=sp)

                    # V: straight evac (cast bf16)
                    nc.scalar.copy(vS[:, tci, :, 0:HD], v_ps[:].rearrange("p (g d) -> p g d", g=KPG))

                    # RMS stats via Square activation with free-axis accumulate
                    q_ps3 = q_ps.rearrange("p (h d) -> p h d", h=HPG)
                    k_ps3 = k_ps.rearrange("p (h d) -> p h d", h=KPG)
                    qss = scr.tile([128, HPG], F32, tag="qss")
                    kss = scr.tile([128, KPG], F32, tag="kss")
                    sqdump = scr.tile([128, 128], BF16, tag="sqdump")
                    for h in range(HPG):
                        nc.scalar.activation(out=sqdump[:], in_=q_ps3[:, h],
                                             func=mybir.ActivationFunctionType.Square,
                                             accum_out=qss[:, h:h + 1])
                    for h in range(KPG):
                        nc.scalar.activation(out=sqdump[:], in_=k_ps3[:, h],
                                             func=mybir.ActivationFunctionType.Square,
                                             accum_out=kss[:, h:h + 1])
                    # rstd = 1/sqrt(ms + eps); fold gain & 1/sqrt(hd) into q scale
                    qstd = scr.tile([128, HPG], F32, tag="qstd")
                    nc.scalar.activation(out=qstd[:], in_=qss[:],
                                         func=mybir.ActivationFunctionType.Sqrt,
                                         scale=1.0 / HD, bias=eps_t[:])
                    qsc = scr.tile([128, HPG], F32, tag="qsc")
                    nc.vector.reciprocal(qsc[:], qstd[:])
                    nc.vector.tensor_mul(qsc[:], qsc[:], gsc_t[:])
                    kstd = scr.tile([128, KPG], F32, tag="kstd")
                    nc.scalar.activation(out=kstd[:], in_=kss[:],
                                         func=mybir.ActivationFunctionType.Sqrt,
                                         scale=1.0 / HD, bias=eps_t[:])
                    ksc = scr.tile([128, KPG], F32, tag="ksc")
                    nc.vector.reciprocal(ksc[:], kstd[:])

                    # scale (broadcast over head_dim), f32 SBUF staging
                    qs = scr.tile([128, HPG, HD], F32, tag="qs")
                    nc.vector.tensor_mul(qs[:], q_ps3, bcast(qsc[:], HD, 2))
                    ks = scr.tile([128, KPG, HD], F32, tag="ks")
                    nc.vector.tensor_mul(ks[:], k_ps3, bcast(ksc[:], HD, 2))

                    # rope (batched over heads; cos/sin broadcast over h)
                    cos_bq = bcast(cos_t[:], HPG, 1)
                    sin_bq = bcast(sin_t[:], HPG, 1)
                    cos_bk = bcast(cos_t[:], KPG, 1)
                    sin_bk = bcast(sin_t[:], KPG, 1)
                    t1 = scr.tile([128, HPG, 64], F32, tag="t1")
                    t2 = scr.tile([128, HPG, 64], F32, tag="t2")
                    qr = scr.tile([128, HPG, 2, 64], BF16, tag="qr")
                    q1 = qs[:, :, 0:64]
                    q2 = qs[:, :, 64:128]
                    nc.vector.tensor_mul(t1[:], q1, cos_bq)
                    nc.vector.tensor_mul(t2[:], q2, sin_bq)
                    nc.vector.tensor_add(qr[:, :, 0, :], t1[:], t2[:])
                    nc.vector.tensor_mul(t1[:], q2, cos_bq)
                    nc.vector.tensor_mul(t2[:], q1, sin_bq)
                    nc.vector.tensor_sub(qr[:, :, 1, :], t1[:], t2[:])
                    k1 = ks[:, :, 0:64]
                    k2 = ks[:, :, 64:128]
                    t1k = scr.tile([128, KPG, 64], F32, tag="t1k")
                    t2k = scr.tile([128, KPG, 64], F32, tag="t2k")
                    kr = scr.tile([128, KPG, 2, 64], BF16, tag="kr")
                    nc.vector.tensor_mul(t1k[:], k1, cos_bk)
                    nc.vector.tensor_mul(t2k[:], k2, sin_bk)
                    nc.vector.tensor_add(kr[:, :, 0, :], t1k[:], t2k[:])
                    nc.vector.tensor_mul(t1k[:], k2, cos_bk)
                    nc.vector.tensor_mul(t2k[:], k1, sin_bk)
                    nc.vector.tensor_sub(kr[:, :, 1, :], t1k[:], t2k[:])

                    # transpose q/k tiles into [d, t] stores (batched evacs)
                    for grp in range(2):
                        tq_ps = psA.tile([128, 512], BF16, tag="trp")
                        for hh in range(4):
                            h = grp * 4 + hh
                            nc.tensor.transpose(tq_ps[:, hh * 128:(hh + 1) * 128],
                                                qr[:, h].rearrange("p a b -> p (a b)"), ident[:])
                        if grp == 0:
                            nc.vector.tensor_copy(qT[:, 0:4, tci, :], tq_ps[:].rearrange("p (a b) -> p a b", a=4))
                        else:
                            nc.scalar.copy(qT[:, 4:8, tci, :], tq_ps[:].rearrange("p (a b) -> p a b", a=4))
                    tk_ps = psA.tile([128, 512], BF16, tag="trp")
                    for h in range(KPG):
                        nc.tensor.transpose(tk_ps[:, h * 128:(h + 1) * 128],
                                            kr[:, h].rearrange("p a b -> p (a b)"), ident[:])
                    nc.vector.tensor_copy(kT[:, 0:KPG, tci, :],
                                          tk_ps[:, 0:KPG * 128].rearrange("p (a b) -> p a b", a=KPG))

            # ---------------- Phases B (flash attention) + C (O-projection)
            with tc_.tile_pool(name="wo", bufs=1) as wop:
                wo_t = wop.tile([128, HPG, DIM], BF16)
                nc.sync.dma_start(out=wo_t, in_=wo.rearrange("cc p o -> p cc o"))

                with tc_.tile_pool(name="ptp", bufs=3) as ptp, \
                     tc_.tile_pool(name="scrB", bufs=2) as scrB, \
                     tc_.tile_pool(name="psB", bufs=2, space="PSUM") as psB, \
                     tc_.tile_pool(name="psB1", bufs=1, space="PSUM") as psB1:
                    SB = 12  # score blocks per PSUM batch (3 banks)
                    for h in range(HPG):
                        kvh = h // (NH // NKV)
                        yt_ps = None
                        for ti in range(TC):
                            nj = ti + 1
                            y_ps = psB1.tile([128, HD + 1], F32, tag="y")
                            for jb in range(0, nj, SB):
                                jbe = min(jb + SB, nj)
                                w = (jbe - jb) * 128
                                s_ps = psB.tile([128, SB * 128], F32, tag="s12")
                                for j in range(jb, jbe):
                                    c0 = (j - jb) * 128
                                    nc.tensor.matmul(s_ps[:, c0:c0 + 128], kT[:, kvh, j, :],
                                                     qT[:, h, ti, :], start=True, stop=True)
                                pt = ptp.tile([128, SB * 128], BF16, tag="pt")
                                nc.scalar.activation(out=pt[:, 0:w], in_=s_ps[:, 0:w],
                                                     func=mybir.ActivationFunctionType.Exp)
                                if ti < jbe:  # diagonal block in this batch: causal mask on P^T
                                    c0 = (ti - jb) * 128
                                    nc.gpsimd.affine_select(
                                        out=pt[:, c0:c0 + 128], in_=pt[:, c0:c0 + 128],
                                        compare_op=mybir.AluOpType.is_ge, fill=0.0,
                                        base=0, pattern=[[1, 128]], channel_multiplier=-1)
                                for j in range(jb, jbe):
                                    c0 = (j - jb) * 128
                                    nc.tensor.matmul(y_ps[:], pt[:, c0:c0 + 128],
                                                     vS[:, j, kvh, :],
                                                     start=(j == 0), stop=(j == ti))
                            rinv = scrB.tile([128, 1], F32, tag="rinv")
                            nc.vector.reciprocal(rinv[:], y_ps[:, HD:HD + 1])
                            y_sb = scrB.tile([128, 128], BF16, tag="ysb")
                            nc.vector.tensor_scalar_mul(y_sb[:], y_ps[:, 0:HD], rinv[:])
                            if ti % 4 == 0:
                                yt_ps = psB1.tile([128, 512], BF16, tag="ytr")
                            nc.tensor.transpose(yt_ps[:, (ti % 4) * 128:(ti % 4 + 1) * 128],
                                                y_sb[:], ident[:])
                            if ti % 4 == 3:
                                nc.vector.tensor_copy(
                                    yT[:, h, ti - 3:ti + 1, :],
                                    yt_ps[:].rearrange("p (a b) -> p a b", a=4))

                # Phase C
                with tc_.tile_pool(name="oC", bufs=2) as oC, \
                     tc_.tile_pool(name="psC", bufs=2, space="PSUM") as psC:
                    for ti in range(TC):
                        o_ps = psC.tile([128, DIM], F32, tag="op")
                        for cc in range(HPG):
                            for oc in range(4):
                                nc.tensor.matmul(o_ps[:, oc * 512:(oc + 1) * 512],
                                                 yT[:, cc, ti, :],
                                                 wo_t[:, cc, oc * 512:(oc + 1) * 512],
                                                 start=(cc == 0), stop=(cc == HPG - 1))
                        o_sb = oC.tile([128, DIM], F32, tag="ob")
                        nc.scalar.copy(o_sb[:, 0:1024], o_ps[:, 0:1024])
                        nc.vector.tensor_copy(o_sb[:, 1024:2048], o_ps[:, 1024:2048])
                        nc.sync.dma_start(out=out_r[ti], in_=o_sb)
        if body_ctx is not None:
            body_ctx.__exit__(None, None, None)

    nc.finalize()
    return nc


# ---------------------------------------------------------------- host side

def _rope_tables():
    inv_freq = ROPE_BASE ** (-np.arange(0, HD, 2, dtype=np.float32) / HD)
    t = np.arange(SEQ, dtype=np.float32)
    freqs = t[:, None] * inv_freq[None, :]
    return np.cos(freqs).astype(np.float32), np.sin(freqs).astype(np.float32)


def make_in_maps(x, Wq, Wk, Wv, Wo, q_gain):
    x = np.asarray(x, np.float32)
    Wq = np.asarray(Wq, np.float32)
    Wk = np.asarray(Wk, np.float32)
    Wv = np.asarray(Wv, np.float32)
    Wo = np.asarray(Wo, np.float32)
    q_gain = np.asarray(q_gain, np.float32)

    cos, sin = _rope_tables()
    cs = np.ascontiguousarray(cos.reshape(TC, 128, 64))
    sn = np.ascontiguousarray(sin.reshape(TC, 128, 64))
    tokz = np.zeros((1, 1), np.float32)

    xts = []
    for b in range(BSZ):
        X5 = x[b].reshape(TC, 128, DC, 128)          # [tc, tt, dc, p]
        xts.append(np.ascontiguousarray(X5.transpose(0, 3, 2, 1)).astype(BF16NP))
    gw = []
    for g in range(2):
        wq_g = np.ascontiguousarray(Wq[g * 1024:(g + 1) * 1024].T).astype(BF16NP).reshape(DC, 128, 1024)
        wk_g = np.ascontiguousarray(Wk[g * 256:(g + 1) * 256].T).astype(BF16NP).reshape(DC, 128, 256)
        wv_g = np.ascontiguousarray(Wv[g * 256:(g + 1) * 256].T).astype(BF16NP).reshape(DC, 128, 256)
        wo_g = np.ascontiguousarray(Wo[:, g * 1024:(g + 1) * 1024].T).astype(BF16NP).reshape(HPG, 128, DIM)
        gsc_g = np.tile((q_gain[g * HPG:(g + 1) * HPG] * SCALE)[None, :], (128, 1)).astype(np.float32)
        gw.append((wq_g, wk_g, wv_g, wo_g, gsc_g))

    in_maps = []
    for c in range(N_CORES):
        b, g = c // 2, c % 2
        wq_g, wk_g, wv_g, wo_g, gsc_g = gw[g]
        in_maps.append(dict(xt=xts[b], wq=wq_g, wk=wk_g, wv=wv_g, wo=wo_g,
                            cs=cs, sn=sn, gsc=gsc_g, tok=tokz))
    return in_maps


# cached compiled runner --------------------------------------------------

_STATE = {}


def _get_runner(loop_iters=None):
    key = loop_iters
    if key in _STATE:
        return _STATE[key]
    import jax
    from jax.sharding import Mesh, PartitionSpec
    try:
        from jax.experimental.shard_map import shard_map
    except ImportError:
        from jax.shard_map import shard_map
    from concourse import bass2jax
    from concourse.bass2jax import _bass_exec_p, install_neuronx_cc_hook, partition_id_tensor

    install_neuronx_cc_hook()
    nc = build_nc(loop_iters)

    partition_name = nc.partition_id_tensor.name if nc.partition_id_tensor else None
    in_names, out_names, out_avals = [], [], []
    for alloc in nc.m.functions[0].allocations:
        if not isinstance(alloc, mybir.MemoryLocationSet):
            continue
        name = alloc.memorylocations[0].name
        if alloc.kind == "ExternalInput":
            if name != partition_name:
                in_names.append(name)
        elif alloc.kind == "ExternalOutput":
            out_names.append(name)
            out_avals.append(jax.core.ShapedArray(tuple(alloc.tensor_shape),
                                                  mybir.dt.np(alloc.dtype)))
    n_params = len(in_names)
    all_in = list(in_names + out_names)
    if partition_name is not None:
        all_in.append(partition_name)
    all_in = tuple(all_in)

    def _body(*args):
        operands = list(args)
        if partition_name is not None:
            operands.append(partition_id_tensor())
        outs = _bass_exec_p.bind(
            *operands,
            out_avals=tuple(out_avals),
            in_names=all_in,
            out_names=tuple(out_names),
            lowering_input_output_aliases=(),
            sim_require_finite=True,
            sim_require_nnan=True,
            nc=nc,
        )
        return tuple(outs)

    devices = jax.devices()[:N_CORES]
    mesh = Mesh(np.asarray(devices), ("core",))
    specs = (PartitionSpec("core"),)
    f = jax.jit(shard_map(_body, mesh=mesh,
                          in_specs=specs * (n_params + len(out_names)),
                          out_specs=specs * len(out_names),
                          check_rep=False),
                keep_unused=True)
    zero_shapes = [(tuple(a.shape), a.dtype) for a in out_avals]
    st = dict(f=f, in_names=in_names, out_names=out_names, zero_shapes=zero_shapes, mesh=mesh)
    _STATE[key] = st
    return st


def run_on_device(in_maps, loop_iters=None, device_args=None):
    """Returns (out_arrays_by_name, device_args) — device_args reusable for re-runs."""
    import jax
    from jax.sharding import NamedSharding, PartitionSpec
    st = _get_runner(loop_iters)
    if device_args is None:
        concat = [np.concatenate([m[n] for m in in_maps], axis=0) for n in st["in_names"]]
        zeros = [np.zeros((N_CORES * s[0],) + tuple(s[1:]), d) for s, d in st["zero_shapes"]]
        sh = NamedSharding(st["mesh"], PartitionSpec("core"))
        device_args = [jax.device_put(a, sh) for a in concat + zeros]
        jax.block_until_ready(device_args)
    outs = st["f"](*device_args)
    return outs, device_args


def kernel(x, Wq, Wk, Wv, Wo, q_gain):
    in_maps = make_in_maps(x, Wq, Wk, Wv, Wo, q_gain)
    outs, _ = run_on_device(in_maps)
    st = _get_runner(None)
    oidx = st["out_names"].index("out")
    full = np.asarray(outs[oidx]).reshape(N_CORES, SEQ, DIM)
    result = np.empty((BSZ, SEQ, DIM), np.float32)
    for b in range(BSZ):
        result[b] = full[2 * b] + full[2 * b + 1]
    return result
